# revision 5
# baseline (speedup 1.0000x reference)
"""Trainium2 Bass kernel for nn_CoAttention.

Sharding: data-parallel over batch. B=16 across 8 cores -> 2 batches/core.
All weights replicated. No collectives.

Precision: fp16 operands everywhere (values are O(1..60)), except the raw
exp(l) tensors (eT/eN) which need bf16 range (values up to ~e^70); all
matmuls accumulate fp32 in PSUM; LSTM cell state kept fp32.

Per-core program (per local batch b):
  h_sT = tanh(W_ref @ src_b.T + b_ref)          [H, S]   MM + ACT(bias)
  h_sN = h_sT.T                                  [S, H]   PE transpose
  eT   = exp(h_r @ h_s.T)                        [R, S]   MM + ACT Exp (Ds fused)
  eN   = eT.T                                    [S, R]   PE transpose
  Ds[r]=sum_s eT (fused accum)   Dr[s]=sum_r eN  (free-dim reduce)
  a_sT = eT/Ds   eS = eN/Dr                      per-partition scalar muls
  c_sT = a_sT.T @ h_rN                           [S, H]
  c_rT = [h_sN | c_sT].T @ eS                    [2H, R]
  xg_d = W_ihT_d.T @ [c_rT; h_rT] + b            [G, R] -> strided scan layout
  LSTM scan fwd+bwd interleaved: gates on partitions, [128,16] PSUM/step,
  xg layout col = 16 t + 2 g + b so each step is one contiguous [128,16] slice.

Outputs return as raw [128, 4R] fp32 SBUF images, decoded on host.
"""

import numpy as np
import ml_dtypes

import concourse.bass as bass
import concourse.mybir as mybir
import concourse.tile as tile
from concourse import bacc
from concourse import bass_utils

BF16 = ml_dtypes.bfloat16
FP16 = np.float16

B, S, R, H = 16, 1024, 512, 512
HD = H // 2          # 256
G = 4 * HD           # 1024
DIN = 3 * H          # 1536
N_CORES = 8
BLOC = B // N_CORES  # 2
SCAN_STEPS = R       # full scan; lower only for dev experiments

F32 = mybir.dt.float32
F16 = mybir.dt.float16
BF = mybir.dt.bfloat16
AF = mybir.ActivationFunctionType
ALU = mybir.AluOpType

_CACHE = {}


def _build_nc():
    nc = bacc.Bacc("TRN2", target_bir_lowering=False, debug=False,
                   num_devices=N_CORES)

    # ---- DRAM I/O (all host-prepped [128, F] SBUF images) ----
    d_srcT = nc.dram_tensor("srcT", [128, BLOC * 4 * S], F16, kind="ExternalInput")
    d_hrT = nc.dram_tensor("hrT", [128, BLOC * 4 * R], F16, kind="ExternalInput")
    d_hrN = nc.dram_tensor("hrN", [128, BLOC * 4 * H], F16, kind="ExternalInput")
    d_wrefT = nc.dram_tensor("wrefT", [128, 4 * H], F16, kind="ExternalInput")
    d_brefT = nc.dram_tensor("brefT", [128, 4], F32, kind="ExternalInput")
    d_wih = {d: nc.dram_tensor(f"wihT_{d}", [128, 12 * G], F16, kind="ExternalInput")
             for d in "fb"}
    d_whh = {d: nc.dram_tensor(f"whhT_{d}", [128, 2 * G], F16, kind="ExternalInput")
             for d in "fb"}
    d_bg = {d: nc.dram_tensor(f"bgT_{d}", [128, 8], F32, kind="ExternalInput")
            for d in "fb"}
    d_id16 = nc.dram_tensor("id16", [128, 128], F16, kind="ExternalInput")
    d_idbf = nc.dram_tensor("idbf", [128, 128], BF, kind="ExternalInput")
    d_out = {d: nc.dram_tensor(f"out_{d}", [128, 4 * R], F32, kind="ExternalOutput")
             for d in "fb"}

    with tile.TileContext(nc) as tc, \
         tc.tile_pool(name="wp", bufs=1) as wp, \
         tc.tile_pool(name="ap", bufs=1) as ap, \
         tc.tile_pool(name="scansb", bufs=4) as scansb, \
         tc.tile_pool(name="pp", bufs=2, space="PSUM") as pp, \
         tc.tile_pool(name="pt", bufs=2, space="PSUM") as pt, \
         tc.tile_pool(name="psc", bufs=2, space="PSUM") as psc:

        # ---- persistent loads ----
        def load(dram, shape, dt):
            t = wp.tile(shape, dt, tag=dram.name, name=dram.name)
            nc.sync.dma_start(t[:], dram[:])
            return t

        hrT = load(d_hrT, [128, BLOC * 4 * R], F16)
        hrN = load(d_hrN, [128, BLOC * 4 * H], F16)
        wrefT = load(d_wrefT, [128, 4 * H], F16)
        brefT = load(d_brefT, [128, 4], F32)
        wih = {d: load(d_wih[d], [128, 12 * G], F16) for d in "fb"}
        whh = {d: load(d_whh[d], [128, 2 * G], F16) for d in "fb"}
        bg = {d: load(d_bg[d], [128, 8], F32) for d in "fb"}
        id16 = load(d_id16, [128, 128], F16)
        idbf = load(d_idbf, [128, 128], BF)

        xg = {d: wp.tile([128, 16 * R], F16, tag=f"xg_{d}", name=f"xg_{d}") for d in "fb"}
        outb = {d: wp.tile([128, 4 * R], F32, tag=f"outsb_{d}", name=f"outsb_{d}") for d in "fb"}

        # ---- attention + xg, per local batch ----
        for b in range(BLOC):
            hrT_b = hrT[:, b * 4 * R:(b + 1) * 4 * R]
            hrN_b = hrN[:, b * 4 * H:(b + 1) * 4 * H]

            srcT_b = ap.tile([128, 4 * S], F16, tag="tagA")
            nc.sync.dma_start(srcT_b[:], d_srcT[:, b * 4 * S:(b + 1) * 4 * S])

            # 1) h_sT [4 Hout-tiles x S]
            hsT = ap.tile([128, 4 * S], F16, tag="tagB")
            for m in range(4):
                for sc in range(2):
                    ps = pp.tile([128, 512], F32, tag="mm")
                    for k in range(4):
                        nc.tensor.matmul(
                            ps[:],
                            wrefT[:, k * H + m * 128: k * H + (m + 1) * 128],
                            srcT_b[:, k * S + sc * 512: k * S + sc * 512 + 512],
                            start=(k == 0), stop=(k == 3))
                    nc.scalar.activation(
                        hsT[:, m * S + sc * 512: m * S + sc * 512 + 512],
                        ps[:], AF.Tanh, bias=brefT[:, m:m + 1])

            # 2) h_sN [8 S-tiles x H] = transpose(h_sT)
            hsN = ap.tile([128, 8 * H], F16, tag="tagC")
            for st in range(8):
                for hc in range(4):
                    pst = pt.tile([128, 128], F16, tag="tr")
                    nc.tensor.transpose(
                        pst[:], hsT[:, hc * S + st * 128: hc * S + st * 128 + 128],
                        id16[:])
                    nc.vector.tensor_copy(
                        hsN[:, st * H + hc * 128: st * H + hc * 128 + 128], pst[:])

            # 3) eT [4 R-tiles x S] = exp(l.T), Ds partials fused into accum_out
            eT = ap.tile([128, 4 * S], BF, tag="tagD")
            ds2 = ap.tile([128, 8], F32, tag="ds2")
            for rt in range(4):
                for sc in range(2):
                    ps = pp.tile([128, 512], F32, tag="mm")
                    for k in range(4):
                        nc.tensor.matmul(
                            ps[:],
                            hrT_b[:, k * R + rt * 128: k * R + (rt + 1) * 128],
                            hsT[:, k * S + sc * 512: k * S + sc * 512 + 512],
                            start=(k == 0), stop=(k == 3))
                    nc.scalar.activation(
                        eT[:, rt * S + sc * 512: rt * S + sc * 512 + 512],
                        ps[:], AF.Exp,
                        accum_out=ds2[:, rt * 2 + sc: rt * 2 + sc + 1])

            # 4) eN [8 S-tiles x R] = transpose(eT)
            eN = ap.tile([128, 8 * R], BF, tag="tagE")
            for st in range(8):
                for rc in range(4):
                    pst = pt.tile([128, 128], BF, tag="trb")
                    nc.tensor.transpose(
                        pst[:], eT[:, rc * S + st * 128: rc * S + st * 128 + 128],
                        idbf[:])
                    nc.vector.tensor_copy(
                        eN[:, st * R + rc * 128: st * R + rc * 128 + 128], pst[:])

            # 5) softmax denominators -> scaled copies (fp16)
            dsum = ap.tile([128, 4], F32, tag="dsum")
            for rt in range(4):
                nc.vector.tensor_add(dsum[:, rt:rt + 1], ds2[:, 2 * rt:2 * rt + 1],
                                     ds2[:, 2 * rt + 1:2 * rt + 2])
            invDs = ap.tile([128, 4], F32, tag="invDs")
            nc.vector.reciprocal(invDs[:], dsum[:])
            drsum = ap.tile([128, 8], F32, tag="drsum")
            for st in range(8):
                nc.vector.tensor_reduce(
                    drsum[:, st:st + 1], eN[:, st * R:(st + 1) * R],
                    mybir.AxisListType.X, ALU.add)
            invDr = ap.tile([128, 8], F32, tag="invDr")
            nc.vector.reciprocal(invDr[:], drsum[:])

            asT = ap.tile([128, 4 * S], F16, tag="tagF")
            for rt in range(4):
                nc.vector.tensor_scalar_mul(
                    asT[:, rt * S:(rt + 1) * S], eT[:, rt * S:(rt + 1) * S],
                    invDs[:, rt:rt + 1])
            eS = ap.tile([128, 8 * R], F16, tag="tagG")
            for st in range(8):
                nc.vector.tensor_scalar_mul(
                    eS[:, st * R:(st + 1) * R], eN[:, st * R:(st + 1) * R],
                    invDr[:, st:st + 1])

            # 6) c_sT [8 S-tiles x H]  (reuses srcT slot)
            csT = ap.tile([128, 8 * H], F16, tag="tagA")
            for st in range(8):
                ps = pp.tile([128, 512], F32, tag="mm")
                for k in range(4):
                    nc.tensor.matmul(
                        ps[:],
                        asT[:, k * S + st * 128: k * S + st * 128 + 128],
                        hrN_b[:, k * H: (k + 1) * H],
                        start=(k == 0), stop=(k == 3))
                nc.vector.tensor_copy(csT[:, st * H:(st + 1) * H], ps[:])

            # 7) c_rT [8 2H-tiles x R]  (reuses hsT slot after last hsT read)
            crT = ap.tile([128, 8 * R], F16, tag="tagB2")
            for m in range(8):
                ps = pp.tile([128, 512], F32, tag="mm")
                for k in range(8):
                    if m < 4:
                        lhsT = hsN[:, k * H + m * 128: k * H + m * 128 + 128]
                    else:
                        lhsT = csT[:, k * H + (m - 4) * 128: k * H + (m - 4) * 128 + 128]
                    nc.tensor.matmul(ps[:], lhsT, eS[:, k * R:(k + 1) * R],
                                     start=(k == 0), stop=(k == 7))
                nc.vector.tensor_copy(crT[:, m * R:(m + 1) * R], ps[:])

            # 8) xg per direction, strided into scan layout (col = 16 t + 2 g + b)
            for d in "fb":
                for g in range(8):
                    ps = pp.tile([128, 512], F32, tag="mm")
                    for k in range(12):
                        if k < 8:
                            rhs = crT[:, k * R:(k + 1) * R]
                        else:
                            rhs = hrT_b[:, (k - 8) * R:(k - 7) * R]
                        nc.tensor.matmul(
                            ps[:],
                            wih[d][:, k * G + g * 128: k * G + (g + 1) * 128],
                            rhs, start=(k == 0), stop=(k == 11))
                    dst = xg[d][:, 2 * g + b:: 16]
                    nc.vector.tensor_scalar_add(dst, ps[:], bg[d][:, g:g + 1])

        # ---- LSTM scan ----
        h_bf = {d: wp.tile([128, 4], F16, tag=f"hbf_{d}", name=f"hbf_{d}") for d in "fb"}
        cst = {d: wp.tile([128, 4], F32, tag=f"c_{d}", name=f"c_{d}") for d in "fb"}
        for d in "fb":
            nc.vector.memset(h_bf[d][:], 0.0)
            nc.vector.memset(cst[d][:], 0.0)

        for t in range(SCAN_STEPS):
            for d in "fb":
                te = t if d == "f" else R - 1 - t
                ps = psc.tile([128, 16], F32, tag="scps")
                for g in range(8):
                    for k in range(2):
                        nc.tensor.matmul(
                            ps[:, 2 * g:2 * g + 2],
                            whh[d][:, k * G + g * 128: k * G + (g + 1) * 128],
                            h_bf[d][:, 2 * k:2 * k + 2],
                            start=(k == 0), stop=(k == 1))
                gp = scansb.tile([128, 16], F32, tag="gp")
                nc.vector.tensor_add(gp[:], ps[:], xg[d][:, 16 * te:16 * te + 16])
                acts = scansb.tile([128, 16], F32, tag="acts")
                nc.scalar.activation(acts[:, 0:8], gp[:, 0:8], AF.Sigmoid)
                nc.scalar.activation(acts[:, 8:12], gp[:, 8:12], AF.Tanh)
                nc.scalar.activation(acts[:, 12:16], gp[:, 12:16], AF.Sigmoid)
                t1 = scansb.tile([128, 4], F32, tag="t1")
                nc.vector.tensor_mul(t1[:], acts[:, 0:4], acts[:, 8:12])
                nc.vector.tensor_mul(cst[d][:], cst[d][:], acts[:, 4:8])
                nc.vector.tensor_add(cst[d][:], cst[d][:], t1[:])
                tc2 = scansb.tile([128, 4], F32, tag="tc2")
                nc.scalar.activation(tc2[:], cst[d][:], AF.Tanh)
                nc.vector.tensor_mul(outb[d][:, 4 * te:4 * te + 4],
                                     tc2[:], acts[:, 12:16])
                nc.vector.tensor_copy(h_bf[d][:], outb[d][:, 4 * te:4 * te + 4])

        for d in "fb":
            nc.sync.dma_start(d_out[d][:], outb[d][:])

    nc.compile()
    return nc


def _img_kmaj(x, p=128):
    """[K, F] -> [128, (K/128)*F] k-tile image."""
    k, f = x.shape
    return np.ascontiguousarray(
        x.reshape(k // p, p, f).transpose(1, 0, 2).reshape(p, (k // p) * f))


def _prep_core(core, inp):
    gb = [BLOC * core + i for i in range(BLOC)]
    src = np.asarray(inp["src_memory_bank"])   # [S, B, H]
    ref = np.asarray(inp["ref_memory_bank"])   # [R, B, H]

    def cat(imgs):
        return np.concatenate(imgs, axis=1)

    m = {}
    m["srcT"] = cat([_img_kmaj(src[:, b, :].T.astype(FP16)) for b in gb])
    m["hrT"] = cat([_img_kmaj(ref[:, b, :].T.astype(FP16)) for b in gb])
    m["hrN"] = cat([_img_kmaj(ref[:, b, :].astype(FP16)) for b in gb])
    m["wrefT"] = _img_kmaj(np.asarray(inp["W_ref"]).T.astype(FP16))
    m["brefT"] = np.ascontiguousarray(
        np.asarray(inp["b_ref"]).astype(np.float32).reshape(4, 128).T)
    for d, sfx in (("f", "_f"), ("b", "_b")):
        m[f"wihT_{d}"] = _img_kmaj(np.asarray(inp[f"W_ih{sfx}"]).T.astype(FP16))
        m[f"whhT_{d}"] = _img_kmaj(np.asarray(inp[f"W_hh{sfx}"]).T.astype(FP16))
        bsum = (np.asarray(inp[f"b_ih{sfx}"], dtype=np.float64)
                + np.asarray(inp[f"b_hh{sfx}"], dtype=np.float64))
        m[f"bgT_{d}"] = np.ascontiguousarray(
            bsum.astype(np.float32).reshape(8, 128).T)
    m["id16"] = np.eye(128, dtype=FP16)
    m["idbf"] = np.eye(128, dtype=BF16)
    return m


def _decode(res_list):
    """results -> [R, B, H] fp32"""
    out = np.zeros((R, B, H), dtype=np.float32)
    for c in range(N_CORES):
        for d, off in (("f", 0), ("b", HD)):
            img = np.asarray(res_list[c][f"out_{d}"])              # [128, 4R]
            x = img.reshape(128, R, 2, BLOC).transpose(1, 3, 2, 0)  # t,b,h,p
            x = np.ascontiguousarray(x).reshape(R, BLOC, HD)
            out[:, BLOC * c:BLOC * (c + 1), off:off + HD] = x
    return out


def kernel(**inputs):
    if "nc" not in _CACHE:
        _CACHE["nc"] = _build_nc()
    nc = _CACHE["nc"]
    in_maps = [_prep_core(c, inputs) for c in range(N_CORES)]
    res = bass_utils.run_bass_kernel_spmd(nc, in_maps,
                                          core_ids=list(range(N_CORES)))
    return _decode(res.results)


# revision 6
# speedup vs baseline: 1.0036x; 1.0036x over previous
"""Trainium2 Bass kernel for nn_CoAttention.

Sharding: data-parallel over batch. B=16 across 8 cores -> 2 batches/core.
All weights replicated. No collectives.

Precision: fp16 operands everywhere (values are O(1..60)), except the raw
exp(l) tensors (eT/eN) which need bf16 range (values up to ~e^70); all
matmuls accumulate fp32 in PSUM; LSTM cell state kept fp32.

Per-core program (per local batch b):
  h_sT = tanh(W_ref @ src_b.T + b_ref)          [H, S]   MM + ACT(bias)
  h_sN = h_sT.T                                  [S, H]   PE transpose
  eT   = exp(h_r @ h_s.T)                        [R, S]   MM + ACT Exp (Ds fused)
  eN   = eT.T                                    [S, R]   PE transpose
  Ds[r]=sum_s eT (fused accum)   Dr[s]=sum_r eN  (free-dim reduce)
  a_sT = eT/Ds   eS = eN/Dr                      per-partition scalar muls
  c_sT = a_sT.T @ h_rN                           [S, H]
  c_rT = [h_sN | c_sT].T @ eS                    [2H, R]
  xg_d = W_ihT_d.T @ [c_rT; h_rT] + b            [G, R] -> strided scan layout
  LSTM scan fwd+bwd interleaved: gates on partitions, [128,16] PSUM/step,
  xg layout col = 16 t + 2 g + b so each step is one contiguous [128,16] slice.

Outputs return as raw [128, 4R] fp32 SBUF images, decoded on host.
"""

import numpy as np
import ml_dtypes

import concourse.bass as bass
import concourse.mybir as mybir
import concourse.tile as tile
from concourse import bacc
from concourse import bass_utils

BF16 = ml_dtypes.bfloat16
FP16 = np.float16

B, S, R, H = 16, 1024, 512, 512
HD = H // 2          # 256
G = 4 * HD           # 1024
DIN = 3 * H          # 1536
N_CORES = 8
BLOC = B // N_CORES  # 2
SCAN_STEPS = R       # full scan; lower only for dev experiments

F32 = mybir.dt.float32
F16 = mybir.dt.float16
BF = mybir.dt.bfloat16
AF = mybir.ActivationFunctionType
ALU = mybir.AluOpType

_CACHE = {}


def _build_nc():
    nc = bacc.Bacc("TRN2", target_bir_lowering=False, debug=False,
                   num_devices=N_CORES)

    # ---- DRAM I/O (all host-prepped [128, F] SBUF images) ----
    d_srcT = nc.dram_tensor("srcT", [128, BLOC * 4 * S], F16, kind="ExternalInput")
    d_hrT = nc.dram_tensor("hrT", [128, BLOC * 4 * R], F16, kind="ExternalInput")
    d_hrN = nc.dram_tensor("hrN", [128, BLOC * 4 * H], F16, kind="ExternalInput")
    d_wrefT = nc.dram_tensor("wrefT", [128, 4 * H], F16, kind="ExternalInput")
    d_brefT = nc.dram_tensor("brefT", [128, 4], F32, kind="ExternalInput")
    d_wih = {d: nc.dram_tensor(f"wihT_{d}", [128, 12 * G], F16, kind="ExternalInput")
             for d in "fb"}
    d_whh = {d: nc.dram_tensor(f"whhT_{d}", [128, 2 * G], F16, kind="ExternalInput")
             for d in "fb"}
    d_bg = {d: nc.dram_tensor(f"bgT_{d}", [128, 8], F32, kind="ExternalInput")
            for d in "fb"}
    d_id16 = nc.dram_tensor("id16", [128, 128], F16, kind="ExternalInput")
    d_idbf = nc.dram_tensor("idbf", [128, 128], BF, kind="ExternalInput")
    d_out = {d: nc.dram_tensor(f"out_{d}", [128, 4 * R], F32, kind="ExternalOutput")
             for d in "fb"}

    with tile.TileContext(nc) as tc, \
         tc.tile_pool(name="wp", bufs=1) as wp, \
         tc.tile_pool(name="ap", bufs=1) as ap, \
         tc.tile_pool(name="scansb", bufs=4) as scansb, \
         tc.tile_pool(name="pp", bufs=2, space="PSUM") as pp, \
         tc.tile_pool(name="pt", bufs=2, space="PSUM") as pt, \
         tc.tile_pool(name="psc", bufs=2, space="PSUM") as psc:

        # ---- persistent loads ----
        def load(dram, shape, dt):
            t = wp.tile(shape, dt, tag=dram.name, name=dram.name)
            nc.sync.dma_start(t[:], dram[:])
            return t

        hrT = load(d_hrT, [128, BLOC * 4 * R], F16)
        hrN = load(d_hrN, [128, BLOC * 4 * H], F16)
        wrefT = load(d_wrefT, [128, 4 * H], F16)
        brefT = load(d_brefT, [128, 4], F32)
        wih = {d: load(d_wih[d], [128, 12 * G], F16) for d in "fb"}
        whh = {d: load(d_whh[d], [128, 2 * G], F16) for d in "fb"}
        bg = {d: load(d_bg[d], [128, 8], F32) for d in "fb"}
        id16 = load(d_id16, [128, 128], F16)
        idbf = load(d_idbf, [128, 128], BF)

        xg = {d: wp.tile([128, 16 * R], F16, tag=f"xg_{d}", name=f"xg_{d}") for d in "fb"}
        outb = {d: wp.tile([128, 4 * R], F32, tag=f"outsb_{d}", name=f"outsb_{d}") for d in "fb"}

        # ---- attention + xg, per local batch ----
        for b in range(BLOC):
            hrT_b = hrT[:, b * 4 * R:(b + 1) * 4 * R]
            hrN_b = hrN[:, b * 4 * H:(b + 1) * 4 * H]

            srcT_b = ap.tile([128, 4 * S], F16, tag="tagA")
            nc.sync.dma_start(srcT_b[:], d_srcT[:, b * 4 * S:(b + 1) * 4 * S])

            # 1) h_sT [4 Hout-tiles x S]
            hsT = ap.tile([128, 4 * S], F16, tag="tagB")
            for m in range(4):
                for sc in range(2):
                    ps = pp.tile([128, 512], F32, tag="mm")
                    for k in range(4):
                        nc.tensor.matmul(
                            ps[:],
                            wrefT[:, k * H + m * 128: k * H + (m + 1) * 128],
                            srcT_b[:, k * S + sc * 512: k * S + sc * 512 + 512],
                            start=(k == 0), stop=(k == 3))
                    nc.scalar.activation(
                        hsT[:, m * S + sc * 512: m * S + sc * 512 + 512],
                        ps[:], AF.Tanh, bias=brefT[:, m:m + 1])

            # 2) h_sN [8 S-tiles x H] = transpose(h_sT)
            hsN = ap.tile([128, 8 * H], F16, tag="tagC")
            for st in range(8):
                for hc in range(4):
                    pst = pt.tile([128, 128], F16, tag="tr")
                    nc.tensor.transpose(
                        pst[:], hsT[:, hc * S + st * 128: hc * S + st * 128 + 128],
                        id16[:])
                    nc.vector.tensor_copy(
                        hsN[:, st * H + hc * 128: st * H + hc * 128 + 128], pst[:])

            # 3) eT [4 R-tiles x S] = exp(l.T), Ds partials fused into accum_out
            eT = ap.tile([128, 4 * S], BF, tag="tagD")
            ds2 = ap.tile([128, 8], F32, tag="ds2")
            for rt in range(4):
                for sc in range(2):
                    ps = pp.tile([128, 512], F32, tag="mm")
                    for k in range(4):
                        nc.tensor.matmul(
                            ps[:],
                            hrT_b[:, k * R + rt * 128: k * R + (rt + 1) * 128],
                            hsT[:, k * S + sc * 512: k * S + sc * 512 + 512],
                            start=(k == 0), stop=(k == 3))
                    nc.scalar.activation(
                        eT[:, rt * S + sc * 512: rt * S + sc * 512 + 512],
                        ps[:], AF.Exp,
                        accum_out=ds2[:, rt * 2 + sc: rt * 2 + sc + 1])

            # 4) eN [8 S-tiles x R] = transpose(eT)
            eN = ap.tile([128, 8 * R], BF, tag="tagE")
            for st in range(8):
                for rc in range(4):
                    pst = pt.tile([128, 128], BF, tag="trb")
                    nc.tensor.transpose(
                        pst[:], eT[:, rc * S + st * 128: rc * S + st * 128 + 128],
                        idbf[:])
                    nc.vector.tensor_copy(
                        eN[:, st * R + rc * 128: st * R + rc * 128 + 128], pst[:])

            # 5) softmax denominators -> scaled copies (fp16)
            dsum = ap.tile([128, 4], F32, tag="dsum")
            for rt in range(4):
                nc.vector.tensor_add(dsum[:, rt:rt + 1], ds2[:, 2 * rt:2 * rt + 1],
                                     ds2[:, 2 * rt + 1:2 * rt + 2])
            invDs = ap.tile([128, 4], F32, tag="invDs")
            nc.vector.reciprocal(invDs[:], dsum[:])
            drsum = ap.tile([128, 8], F32, tag="drsum")
            for st in range(8):
                nc.vector.tensor_reduce(
                    drsum[:, st:st + 1], eN[:, st * R:(st + 1) * R],
                    mybir.AxisListType.X, ALU.add)
            invDr = ap.tile([128, 8], F32, tag="invDr")
            nc.vector.reciprocal(invDr[:], drsum[:])

            asT = ap.tile([128, 4 * S], F16, tag="tagF")
            for rt in range(4):
                nc.vector.tensor_scalar_mul(
                    asT[:, rt * S:(rt + 1) * S], eT[:, rt * S:(rt + 1) * S],
                    invDs[:, rt:rt + 1])
            eS = ap.tile([128, 8 * R], F16, tag="tagG")
            for st in range(8):
                nc.vector.tensor_scalar_mul(
                    eS[:, st * R:(st + 1) * R], eN[:, st * R:(st + 1) * R],
                    invDr[:, st:st + 1])

            # 6) c_sT [8 S-tiles x H]  (reuses srcT slot)
            csT = ap.tile([128, 8 * H], F16, tag="tagA")
            for st in range(8):
                ps = pp.tile([128, 512], F32, tag="mm")
                for k in range(4):
                    nc.tensor.matmul(
                        ps[:],
                        asT[:, k * S + st * 128: k * S + st * 128 + 128],
                        hrN_b[:, k * H: (k + 1) * H],
                        start=(k == 0), stop=(k == 3))
                nc.vector.tensor_copy(csT[:, st * H:(st + 1) * H], ps[:])

            # 7) c_rT [8 2H-tiles x R]  (reuses hsT slot after last hsT read)
            crT = ap.tile([128, 8 * R], F16, tag="tagB2")
            for m in range(8):
                ps = pp.tile([128, 512], F32, tag="mm")
                for k in range(8):
                    if m < 4:
                        lhsT = hsN[:, k * H + m * 128: k * H + m * 128 + 128]
                    else:
                        lhsT = csT[:, k * H + (m - 4) * 128: k * H + (m - 4) * 128 + 128]
                    nc.tensor.matmul(ps[:], lhsT, eS[:, k * R:(k + 1) * R],
                                     start=(k == 0), stop=(k == 7))
                nc.vector.tensor_copy(crT[:, m * R:(m + 1) * R], ps[:])

            # 8) xg per direction, strided into scan layout (col = 16 t + 2 g + b)
            for d in "fb":
                for g in range(8):
                    ps = pp.tile([128, 512], F32, tag="mm")
                    for k in range(12):
                        if k < 8:
                            rhs = crT[:, k * R:(k + 1) * R]
                        else:
                            rhs = hrT_b[:, (k - 8) * R:(k - 7) * R]
                        nc.tensor.matmul(
                            ps[:],
                            wih[d][:, k * G + g * 128: k * G + (g + 1) * 128],
                            rhs, start=(k == 0), stop=(k == 11))
                    dst = xg[d][:, 2 * g + b:: 16]
                    nc.vector.tensor_scalar_add(dst, ps[:], bg[d][:, g:g + 1])

        # ---- LSTM scan ----
        h_bf = {d: wp.tile([128, 4], F16, tag=f"hbf_{d}", name=f"hbf_{d}") for d in "fb"}
        cst = {d: wp.tile([128, 4], F32, tag=f"c_{d}", name=f"c_{d}") for d in "fb"}
        for d in "fb":
            nc.vector.memset(h_bf[d][:], 0.0)
            nc.vector.memset(cst[d][:], 0.0)
            if SCAN_STEPS < R:
                nc.vector.memset(outb[d][:], 0.0)

        for t in range(SCAN_STEPS):
            for d in "fb":
                te = t if d == "f" else R - 1 - t
                ps = psc.tile([128, 16], F32, tag="scps")
                for g in range(8):
                    for k in range(2):
                        nc.tensor.matmul(
                            ps[:, 2 * g:2 * g + 2],
                            whh[d][:, k * G + g * 128: k * G + (g + 1) * 128],
                            h_bf[d][:, 2 * k:2 * k + 2],
                            start=(k == 0), stop=(k == 1))
                gp = scansb.tile([128, 16], F32, tag="gp")
                nc.vector.tensor_add(gp[:], ps[:], xg[d][:, 16 * te:16 * te + 16])
                acts = scansb.tile([128, 16], F32, tag="acts")
                nc.scalar.activation(acts[:, 0:8], gp[:, 0:8], AF.Sigmoid)
                nc.scalar.activation(acts[:, 8:12], gp[:, 8:12], AF.Tanh)
                nc.scalar.activation(acts[:, 12:16], gp[:, 12:16], AF.Sigmoid)
                t1 = scansb.tile([128, 4], F32, tag="t1")
                nc.vector.tensor_mul(t1[:], acts[:, 0:4], acts[:, 8:12])
                nc.vector.tensor_mul(cst[d][:], cst[d][:], acts[:, 4:8])
                nc.vector.tensor_add(cst[d][:], cst[d][:], t1[:])
                tc2 = scansb.tile([128, 4], F32, tag="tc2")
                nc.scalar.activation(tc2[:], cst[d][:], AF.Tanh)
                nc.vector.tensor_mul(outb[d][:, 4 * te:4 * te + 4],
                                     tc2[:], acts[:, 12:16])
                nc.vector.tensor_copy(h_bf[d][:], outb[d][:, 4 * te:4 * te + 4])

        for d in "fb":
            nc.sync.dma_start(d_out[d][:], outb[d][:])

    nc.compile()
    return nc


def _img_kmaj(x, p=128):
    """[K, F] -> [128, (K/128)*F] k-tile image."""
    k, f = x.shape
    return np.ascontiguousarray(
        x.reshape(k // p, p, f).transpose(1, 0, 2).reshape(p, (k // p) * f))


def _prep_core(core, inp):
    gb = [BLOC * core + i for i in range(BLOC)]
    src = np.asarray(inp["src_memory_bank"])   # [S, B, H]
    ref = np.asarray(inp["ref_memory_bank"])   # [R, B, H]

    def cat(imgs):
        return np.concatenate(imgs, axis=1)

    m = {}
    m["srcT"] = cat([_img_kmaj(src[:, b, :].T.astype(FP16)) for b in gb])
    m["hrT"] = cat([_img_kmaj(ref[:, b, :].T.astype(FP16)) for b in gb])
    m["hrN"] = cat([_img_kmaj(ref[:, b, :].astype(FP16)) for b in gb])
    m["wrefT"] = _img_kmaj(np.asarray(inp["W_ref"]).T.astype(FP16))
    m["brefT"] = np.ascontiguousarray(
        np.asarray(inp["b_ref"]).astype(np.float32).reshape(4, 128).T)
    for d, sfx in (("f", "_f"), ("b", "_b")):
        m[f"wihT_{d}"] = _img_kmaj(np.asarray(inp[f"W_ih{sfx}"]).T.astype(FP16))
        m[f"whhT_{d}"] = _img_kmaj(np.asarray(inp[f"W_hh{sfx}"]).T.astype(FP16))
        bsum = (np.asarray(inp[f"b_ih{sfx}"], dtype=np.float64)
                + np.asarray(inp[f"b_hh{sfx}"], dtype=np.float64))
        m[f"bgT_{d}"] = np.ascontiguousarray(
            bsum.astype(np.float32).reshape(8, 128).T)
    m["id16"] = np.eye(128, dtype=FP16)
    m["idbf"] = np.eye(128, dtype=BF16)
    return m


def _decode(res_list):
    """results -> [R, B, H] fp32"""
    out = np.zeros((R, B, H), dtype=np.float32)
    for c in range(N_CORES):
        for d, off in (("f", 0), ("b", HD)):
            img = np.asarray(res_list[c][f"out_{d}"])              # [128, 4R]
            x = img.reshape(128, R, 2, BLOC).transpose(1, 3, 2, 0)  # t,b,h,p
            x = np.ascontiguousarray(x).reshape(R, BLOC, HD)
            out[:, BLOC * c:BLOC * (c + 1), off:off + HD] = x
    return out


def kernel(**inputs):
    if "nc" not in _CACHE:
        _CACHE["nc"] = _build_nc()
    nc = _CACHE["nc"]
    in_maps = [_prep_core(c, inputs) for c in range(N_CORES)]
    res = bass_utils.run_bass_kernel_spmd(nc, in_maps,
                                          core_ids=list(range(N_CORES)))
    return _decode(res.results)


# revision 7
# speedup vs baseline: 1.0218x; 1.0181x over previous
"""Trainium2 Bass kernel for nn_CoAttention.

Sharding: data-parallel over batch. B=16 across 8 cores -> 2 batches/core.
All weights replicated. No collectives.

Precision: fp16 operands everywhere (values are O(1..60)), except the raw
exp(l) tensors (eT/eN) which need bf16 range (values up to ~e^70); all
matmuls accumulate fp32 in PSUM; LSTM cell state kept fp32.

Per-core program (per local batch b):
  h_sT = tanh(W_ref @ src_b.T + b_ref)          [H, S]   MM + ACT(bias)
  h_sN = h_sT.T                                  [S, H]   PE transpose
  eT   = exp(h_r @ h_s.T)                        [R, S]   MM + ACT Exp (Ds fused)
  eN   = eT.T                                    [S, R]   PE transpose
  Ds[r]=sum_s eT (fused accum)   Dr[s]=sum_r eN  (free-dim reduce)
  a_sT = eT/Ds   eS = eN/Dr                      per-partition scalar muls
  c_sT = a_sT.T @ h_rN                           [S, H]
  c_rT = [h_sN | c_sT].T @ eS                    [2H, R]
  xg_d = W_ihT_d.T @ [c_rT; h_rT] + b            [G, R] -> strided scan layout
  LSTM scan fwd+bwd interleaved: gates on partitions, [128,16] PSUM/step,
  xg layout col = 16 t + 2 g + b so each step is one contiguous [128,16] slice.

Outputs return as raw [128, 4R] fp32 SBUF images, decoded on host.
"""

import numpy as np
import ml_dtypes

import concourse.bass as bass
import concourse.mybir as mybir
import concourse.tile as tile
from concourse import bacc
from concourse import bass_utils

BF16 = ml_dtypes.bfloat16
FP16 = np.float16

B, S, R, H = 16, 1024, 512, 512
HD = H // 2          # 256
G = 4 * HD           # 1024
DIN = 3 * H          # 1536
N_CORES = 8
BLOC = B // N_CORES  # 2
SCAN_STEPS = R       # full scan; lower only for dev experiments
XG_PRELOAD = False   # DVE-preload xg into PSUM, matmuls accumulate onto it
# Gate-type permutation (host side): blocks reordered i,f,o,g so sigmoid gates
# are contiguous -> 2 ACT ops per scan step instead of 3.
_GPERM = np.r_[0:512, 768:1024, 512:768]

F32 = mybir.dt.float32
F16 = mybir.dt.float16
BF = mybir.dt.bfloat16
AF = mybir.ActivationFunctionType
ALU = mybir.AluOpType

_CACHE = {}


def _build_nc():
    nc = bacc.Bacc("TRN2", target_bir_lowering=False, debug=False,
                   num_devices=N_CORES)

    # ---- DRAM I/O (all host-prepped [128, F] SBUF images) ----
    d_srcT = nc.dram_tensor("srcT", [128, BLOC * 4 * S], F16, kind="ExternalInput")
    d_hrT = nc.dram_tensor("hrT", [128, BLOC * 4 * R], F16, kind="ExternalInput")
    d_hrN = nc.dram_tensor("hrN", [128, BLOC * 4 * H], F16, kind="ExternalInput")
    d_wrefT = nc.dram_tensor("wrefT", [128, 4 * H], F16, kind="ExternalInput")
    d_brefT = nc.dram_tensor("brefT", [128, 4], F32, kind="ExternalInput")
    d_wih = {d: nc.dram_tensor(f"wihT_{d}", [128, 12 * G], F16, kind="ExternalInput")
             for d in "fb"}
    d_whh = {d: nc.dram_tensor(f"whhT_{d}", [128, 2 * G], F16, kind="ExternalInput")
             for d in "fb"}
    d_bg = {d: nc.dram_tensor(f"bgT_{d}", [128, 8], F32, kind="ExternalInput")
            for d in "fb"}
    d_id16 = nc.dram_tensor("id16", [128, 128], F16, kind="ExternalInput")
    d_idbf = nc.dram_tensor("idbf", [128, 128], BF, kind="ExternalInput")
    d_out = {d: nc.dram_tensor(f"out_{d}", [128, 4 * R], F32, kind="ExternalOutput")
             for d in "fb"}

    with tile.TileContext(nc) as tc, \
         tc.tile_pool(name="wp", bufs=1) as wp, \
         tc.tile_pool(name="ap", bufs=1) as ap, \
         tc.tile_pool(name="scansb", bufs=8) as scansb, \
         tc.tile_pool(name="pp", bufs=2, space="PSUM") as pp, \
         tc.tile_pool(name="pt", bufs=1, space="PSUM") as pt, \
         tc.tile_pool(name="psc", bufs=3, space="PSUM") as psc:

        # ---- persistent loads ----
        def load(dram, shape, dt):
            t = wp.tile(shape, dt, tag=dram.name, name=dram.name)
            nc.sync.dma_start(t[:], dram[:])
            return t

        hrT = load(d_hrT, [128, BLOC * 4 * R], F16)
        hrN = load(d_hrN, [128, BLOC * 4 * H], F16)
        wrefT = load(d_wrefT, [128, 4 * H], F16)
        brefT = load(d_brefT, [128, 4], F32)
        wih = {d: load(d_wih[d], [128, 12 * G], F16) for d in "fb"}
        whh = {d: load(d_whh[d], [128, 2 * G], F16) for d in "fb"}
        bg = {d: load(d_bg[d], [128, 8], F32) for d in "fb"}
        id16 = load(d_id16, [128, 128], F16)
        idbf = load(d_idbf, [128, 128], BF)

        xg = {d: wp.tile([128, 16 * R], F16, tag=f"xg_{d}", name=f"xg_{d}") for d in "fb"}
        outb = {d: wp.tile([128, 4 * R], F32, tag=f"outsb_{d}", name=f"outsb_{d}") for d in "fb"}

        # ---- attention + xg, per local batch ----
        for b in range(BLOC):
            hrT_b = hrT[:, b * 4 * R:(b + 1) * 4 * R]
            hrN_b = hrN[:, b * 4 * H:(b + 1) * 4 * H]

            srcT_b = ap.tile([128, 4 * S], F16, tag="tagA")
            nc.sync.dma_start(srcT_b[:], d_srcT[:, b * 4 * S:(b + 1) * 4 * S])

            # 1) h_sT [4 Hout-tiles x S]
            hsT = ap.tile([128, 4 * S], F16, tag="tagB")
            for m in range(4):
                for sc in range(2):
                    ps = pp.tile([128, 512], F32, tag="mm")
                    for k in range(4):
                        nc.tensor.matmul(
                            ps[:],
                            wrefT[:, k * H + m * 128: k * H + (m + 1) * 128],
                            srcT_b[:, k * S + sc * 512: k * S + sc * 512 + 512],
                            start=(k == 0), stop=(k == 3))
                    nc.scalar.activation(
                        hsT[:, m * S + sc * 512: m * S + sc * 512 + 512],
                        ps[:], AF.Tanh, bias=brefT[:, m:m + 1])

            # 2) h_sN [8 S-tiles x H] = transpose(h_sT)
            hsN = ap.tile([128, 8 * H], F16, tag="tagC")
            for st in range(8):
                for hc in range(4):
                    pst = pt.tile([128, 128], F16, tag="tr")
                    nc.tensor.transpose(
                        pst[:], hsT[:, hc * S + st * 128: hc * S + st * 128 + 128],
                        id16[:])
                    nc.vector.tensor_copy(
                        hsN[:, st * H + hc * 128: st * H + hc * 128 + 128], pst[:])

            # 3) eT [4 R-tiles x S] = exp(l.T), Ds partials fused into accum_out
            eT = ap.tile([128, 4 * S], BF, tag="tagD")
            ds2 = ap.tile([128, 8], F32, tag="ds2")
            for rt in range(4):
                for sc in range(2):
                    ps = pp.tile([128, 512], F32, tag="mm")
                    for k in range(4):
                        nc.tensor.matmul(
                            ps[:],
                            hrT_b[:, k * R + rt * 128: k * R + (rt + 1) * 128],
                            hsT[:, k * S + sc * 512: k * S + sc * 512 + 512],
                            start=(k == 0), stop=(k == 3))
                    nc.scalar.activation(
                        eT[:, rt * S + sc * 512: rt * S + sc * 512 + 512],
                        ps[:], AF.Exp,
                        accum_out=ds2[:, rt * 2 + sc: rt * 2 + sc + 1])

            # 4) eN [8 S-tiles x R] = transpose(eT)
            eN = ap.tile([128, 8 * R], BF, tag="tagE")
            for st in range(8):
                for rc in range(4):
                    pst = pt.tile([128, 128], BF, tag="trb")
                    nc.tensor.transpose(
                        pst[:], eT[:, rc * S + st * 128: rc * S + st * 128 + 128],
                        idbf[:])
                    nc.vector.tensor_copy(
                        eN[:, st * R + rc * 128: st * R + rc * 128 + 128], pst[:])

            # 5) softmax denominators -> scaled copies (fp16)
            dsum = ap.tile([128, 4], F32, tag="dsum")
            for rt in range(4):
                nc.vector.tensor_add(dsum[:, rt:rt + 1], ds2[:, 2 * rt:2 * rt + 1],
                                     ds2[:, 2 * rt + 1:2 * rt + 2])
            invDs = ap.tile([128, 4], F32, tag="invDs")
            nc.vector.reciprocal(invDs[:], dsum[:])
            drsum = ap.tile([128, 8], F32, tag="drsum")
            for st in range(8):
                nc.vector.tensor_reduce(
                    drsum[:, st:st + 1], eN[:, st * R:(st + 1) * R],
                    mybir.AxisListType.X, ALU.add)
            invDr = ap.tile([128, 8], F32, tag="invDr")
            nc.vector.reciprocal(invDr[:], drsum[:])

            asT = ap.tile([128, 4 * S], F16, tag="tagF")
            for rt in range(4):
                nc.vector.tensor_scalar_mul(
                    asT[:, rt * S:(rt + 1) * S], eT[:, rt * S:(rt + 1) * S],
                    invDs[:, rt:rt + 1])
            eS = ap.tile([128, 8 * R], F16, tag="tagG")
            for st in range(8):
                nc.vector.tensor_scalar_mul(
                    eS[:, st * R:(st + 1) * R], eN[:, st * R:(st + 1) * R],
                    invDr[:, st:st + 1])

            # 6) c_sT [8 S-tiles x H]  (reuses srcT slot)
            csT = ap.tile([128, 8 * H], F16, tag="tagA")
            for st in range(8):
                ps = pp.tile([128, 512], F32, tag="mm")
                for k in range(4):
                    nc.tensor.matmul(
                        ps[:],
                        asT[:, k * S + st * 128: k * S + st * 128 + 128],
                        hrN_b[:, k * H: (k + 1) * H],
                        start=(k == 0), stop=(k == 3))
                nc.vector.tensor_copy(csT[:, st * H:(st + 1) * H], ps[:])

            # 7) c_rT [8 2H-tiles x R]  (reuses hsT slot after last hsT read)
            crT = ap.tile([128, 8 * R], F16, tag="tagB2")
            for m in range(8):
                ps = pp.tile([128, 512], F32, tag="mm")
                for k in range(8):
                    if m < 4:
                        lhsT = hsN[:, k * H + m * 128: k * H + m * 128 + 128]
                    else:
                        lhsT = csT[:, k * H + (m - 4) * 128: k * H + (m - 4) * 128 + 128]
                    nc.tensor.matmul(ps[:], lhsT, eS[:, k * R:(k + 1) * R],
                                     start=(k == 0), stop=(k == 7))
                nc.vector.tensor_copy(crT[:, m * R:(m + 1) * R], ps[:])

            # 8) xg per direction, strided into scan layout (col = 16 t + 2 g + b)
            for d in "fb":
                for g in range(8):
                    ps = pp.tile([128, 512], F32, tag="mm")
                    for k in range(12):
                        if k < 8:
                            rhs = crT[:, k * R:(k + 1) * R]
                        else:
                            rhs = hrT_b[:, (k - 8) * R:(k - 7) * R]
                        nc.tensor.matmul(
                            ps[:],
                            wih[d][:, k * G + g * 128: k * G + (g + 1) * 128],
                            rhs, start=(k == 0), stop=(k == 11))
                    dst = xg[d][:, 2 * g + b:: 16]
                    nc.vector.tensor_scalar_add(dst, ps[:], bg[d][:, g:g + 1])

        # ---- LSTM scan ----
        h_bf = {d: wp.tile([128, 4], F16, tag=f"hbf_{d}", name=f"hbf_{d}") for d in "fb"}
        cst = {d: wp.tile([128, 4], F32, tag=f"c_{d}", name=f"c_{d}") for d in "fb"}
        for d in "fb":
            nc.vector.memset(h_bf[d][:], 0.0)
            nc.vector.memset(cst[d][:], 0.0)
            if SCAN_STEPS < R:
                nc.vector.memset(outb[d][:], 0.0)

        for t in range(SCAN_STEPS):
            for d in "fb":
                te = t if d == "f" else R - 1 - t
                ps = psc.tile([128, 16], F32, tag="scps")
                if XG_PRELOAD:
                    nc.vector.tensor_copy(ps[:], xg[d][:, 16 * te:16 * te + 16])
                for g in range(8):
                    for k in range(2):
                        nc.tensor.matmul(
                            ps[:, 2 * g:2 * g + 2],
                            whh[d][:, k * G + g * 128: k * G + (g + 1) * 128],
                            h_bf[d][:, 2 * k:2 * k + 2],
                            start=(k == 0 and not XG_PRELOAD), stop=(k == 1),
                            skip_group_check=XG_PRELOAD)
                if XG_PRELOAD:
                    gp = ps
                else:
                    gp = scansb.tile([128, 16], F32, tag="gp")
                    nc.vector.tensor_add(gp[:], ps[:],
                                         xg[d][:, 16 * te:16 * te + 16])
                # gate order (host-permuted): i 0:4, f 4:8, o 8:12, g 12:16
                acts = scansb.tile([128, 16], F32, tag="acts")
                nc.scalar.activation(acts[:, 0:12], gp[:, 0:12], AF.Sigmoid)
                nc.scalar.activation(acts[:, 12:16], gp[:, 12:16], AF.Tanh)
                t1 = scansb.tile([128, 4], F32, tag="t1")
                nc.vector.tensor_mul(t1[:], acts[:, 0:4], acts[:, 12:16])
                nc.vector.tensor_mul(cst[d][:], cst[d][:], acts[:, 4:8])
                nc.vector.tensor_add(cst[d][:], cst[d][:], t1[:])
                tc2 = scansb.tile([128, 4], F32, tag="tc2")
                nc.scalar.activation(tc2[:], cst[d][:], AF.Tanh)
                nc.vector.tensor_mul(h_bf[d][:], tc2[:], acts[:, 8:12])
                nc.vector.tensor_mul(outb[d][:, 4 * te:4 * te + 4],
                                     tc2[:], acts[:, 8:12])

        for d in "fb":
            nc.sync.dma_start(d_out[d][:], outb[d][:])

    nc.compile()
    return nc


def _img_kmaj(x, p=128):
    """[K, F] -> [128, (K/128)*F] k-tile image."""
    k, f = x.shape
    return np.ascontiguousarray(
        x.reshape(k // p, p, f).transpose(1, 0, 2).reshape(p, (k // p) * f))


def _prep_core(core, inp):
    gb = [BLOC * core + i for i in range(BLOC)]
    src = np.asarray(inp["src_memory_bank"])   # [S, B, H]
    ref = np.asarray(inp["ref_memory_bank"])   # [R, B, H]

    def cat(imgs):
        return np.concatenate(imgs, axis=1)

    m = {}
    m["srcT"] = cat([_img_kmaj(src[:, b, :].T.astype(FP16)) for b in gb])
    m["hrT"] = cat([_img_kmaj(ref[:, b, :].T.astype(FP16)) for b in gb])
    m["hrN"] = cat([_img_kmaj(ref[:, b, :].astype(FP16)) for b in gb])
    m["wrefT"] = _img_kmaj(np.asarray(inp["W_ref"]).T.astype(FP16))
    m["brefT"] = np.ascontiguousarray(
        np.asarray(inp["b_ref"]).astype(np.float32).reshape(4, 128).T)
    for d, sfx in (("f", "_f"), ("b", "_b")):
        m[f"wihT_{d}"] = _img_kmaj(
            np.asarray(inp[f"W_ih{sfx}"])[_GPERM].T.astype(FP16))
        m[f"whhT_{d}"] = _img_kmaj(
            np.asarray(inp[f"W_hh{sfx}"])[_GPERM].T.astype(FP16))
        bsum = (np.asarray(inp[f"b_ih{sfx}"], dtype=np.float64)
                + np.asarray(inp[f"b_hh{sfx}"], dtype=np.float64))[_GPERM]
        m[f"bgT_{d}"] = np.ascontiguousarray(
            bsum.astype(np.float32).reshape(8, 128).T)
    m["id16"] = np.eye(128, dtype=FP16)
    m["idbf"] = np.eye(128, dtype=BF16)
    return m


def _decode(res_list):
    """results -> [R, B, H] fp32"""
    out = np.zeros((R, B, H), dtype=np.float32)
    for c in range(N_CORES):
        for d, off in (("f", 0), ("b", HD)):
            img = np.asarray(res_list[c][f"out_{d}"])              # [128, 4R]
            x = img.reshape(128, R, 2, BLOC).transpose(1, 3, 2, 0)  # t,b,h,p
            x = np.ascontiguousarray(x).reshape(R, BLOC, HD)
            out[:, BLOC * c:BLOC * (c + 1), off:off + HD] = x
    return out


def kernel(**inputs):
    if "nc" not in _CACHE:
        _CACHE["nc"] = _build_nc()
    nc = _CACHE["nc"]
    in_maps = [_prep_core(c, inputs) for c in range(N_CORES)]
    res = bass_utils.run_bass_kernel_spmd(nc, in_maps,
                                          core_ids=list(range(N_CORES)))
    return _decode(res.results)
